# revision 6
# baseline (speedup 1.0000x reference)
"""ConceptContrastiveLoss Trainium2 kernel (8-core SPMD, batch-parallel).

Takes FULL inputs expert_concepts/violator_concepts [256, 2048, 128] f32,
returns the scalar loss. Internally shards the batch dim across 8 cores.

Per core (64 MiB of input): each batch item [2048, 128] is DMA'd as one
contiguous 1 MiB transfer into SBUF [128 partitions x 2048] (16 seq rows
per partition).  One VectorE tensor_add folds the halves fp32->fp16
([128, 1024], exact-in-fp32 add, one fp16 rounding per element), then 8
fp16 ones-matmuls on TensorE fold the partitions into a D-major centroid
column accumulated in PSUM.  fp16 keeps the DVE pass at 1 elem/cycle
input-limited (~1.2 us/tile vs ~2.6 us for the fp32 fold tree, which was
the previous bottleneck: 64 tiles x 2.6 us = 168 us DVE-busy vs the
~150 us DMA floor) and makes the partition-fold matmuls 1 cycle/row
instead of fp32's 4.

The tiny [128, 64] fp16 centroid block is AllGathered; every core then
computes the pairwise separation/clustering terms redundantly in fp16
(-2*X^T*Y matmuls plus a rank-2 norms matmul, ACT sqrt/relu/square with
accum_out reductions in fp32) and core 0's scalar is returned.

fp16 (10 mantissa bits) over bf16: centroid-sum quantization error ~4x
smaller; all stored magnitudes (pair-sums <~10, centroids ~0.02, squares
~5e-4) sit comfortably in fp16 normal range.
"""

import numpy as np

import concourse.bacc as bacc
import concourse.bass as bass
import concourse.mybir as mybir
import concourse.tile as tile
from concourse.bass_utils import run_bass_kernel_spmd
from concourse.tile import add_dep_helper

F32 = mybir.dt.float32
F16 = mybir.dt.float16

MARGIN = 10.0
ALPHA = 3.0
BETA = 0.3
GAMMA = 0.3

B, S, D = 256, 2048, 128
N_CORES = 8
BPD = 1   # batch items per DMA (DMA size = BPD MiB)
BUFS = 8  # big-tile pool buffers
N_DMA_ENG = 2  # DMA issue streams: 2 = SP+ACT HWDGE, 3 = + gpsimd SWDGE
F16_STOP = 1024  # post-fold width per batch item (1024 => 1 DVE fold, 8 mm)


def _build_body(tc, e, v, out, loc_cent, gath, B, S, D, n_cores, solo=False,
                bpd=BPD, bufs=BUFS, loop_r=1, f16_stop=None, n_dma_eng=None,
                start_dep=None, rep=0):
    nc = tc.nc
    if f16_stop is None:
        f16_stop = F16_STOP
    if n_dma_eng is None:
        n_dma_eng = N_DMA_ENG
    Bl = B // n_cores  # local batches per tensor
    J = S // 128       # seq tiles per batch item
    n_pairs = B * (B - 1) // 2
    w_ev = ALPHA / (B * B)
    w_ee = BETA / (2.0 * n_pairs)
    w_vv = GAMMA / (2.0 * n_pairs)
    blks = [(s, min(128, B - s)) for s in range(0, B, 128)]
    n_blk = len(blks)

    from contextlib import ExitStack

    sfx = f"_{rep}" if rep else ""
    with ExitStack() as ctx:
        consts = ctx.enter_context(tc.tile_pool(name="consts" + sfx, bufs=1))
        # preamble-initialized const (no Tile dep => no extra sem wait on
        # matmuls)
        ones_col = nc.const_aps.aps[(F32, 1.0)]
        b_eps = consts.tile([128, 1], F32, name="b_eps")
        nc.vector.memset(b_eps[:], 1e-12)
        b_margin = consts.tile([128, 1], F32, name="b_margin")
        nc.vector.memset(b_margin[:], MARGIN)
        ones16 = consts.tile([128, 1], F16, name="ones16")
        nc.vector.memset(ones16[:], 1.0)
        centS = consts.tile([D, 2 * Bl], F16, name="centS")

        # ---- bulk phase: per-batch centroid sums ----
        # 8 batch columns share one PSUM bank (one accumulation group), so
        # all 8 pool bufs are used exactly once -- no slot reuse, and every
        # matmul carries at most the single DMA/DVE wait it needs.
        cent_copies = []
        GRP = min(8, 2 * Bl)
        n_groups = (2 * Bl + GRP - 1) // GRP
        assert GRP % bpd == 0 and Bl % bpd == 0
        W2 = J * D // 2          # post-fold width per batch item (1024)
        n_folds = f16_stop // D  # matmuls per batch item
        with (
            tc.tile_pool(name="big" + sfx, bufs=bufs) as big_pool,
            tc.tile_pool(name="b16" + sfx, bufs=bufs) as b16_pool,
            tc.tile_pool(name="cps" + sfx, bufs=8, space="PSUM") as cps,
        ):
            dma_engines = [nc.sync, nc.scalar, nc.gpsimd][:n_dma_eng]

            def emit_bulk():
                dma_i = 0
                first_dma = None
                for g in range(n_groups):
                    G = cps.tile([128, 512], F32, name="Gacc")
                    start_mm = None
                    col_last = []
                    for ti in range(GRP // bpd):
                        gi0 = g * GRP + ti * bpd
                        t_idx, b0 = divmod(gi0, Bl)
                        src = (e, v)[t_idx]
                        Tb = big_pool.tile([128, bpd * J * D], F32, name="Tb")
                        T16 = b16_pool.tile([128, bpd * W2], F16, name="T16")
                        eng = dma_engines[dma_i % len(dma_engines)]
                        dma_i += 1
                        if bpd == 1:
                            dma = eng.dma_start(
                                out=Tb[:],
                                in_=src[b0].rearrange("(p j) d -> p (j d)", p=128),
                            )
                        else:
                            dma = eng.dma_start(
                                out=Tb[:].rearrange("p (b x) -> p b x", b=bpd),
                                in_=src[b0 : b0 + bpd].rearrange(
                                    "b (p j) d -> p b (j d)", p=128
                                ),
                            )
                        if first_dma is None:
                            first_dma = dma
                            if start_dep is not None:
                                add_dep_helper(
                                    dma.ins, start_dep.ins, sync=True,
                                    reason="serialize repeats",
                                )
                        # one fold: fp32 halves add -> fp16 (exact fp32 add,
                        # single fp16 rounding); covers all bpd batches via
                        # 3D APs to amortize DVE instruction overhead
                        if bpd == 1:
                            nc.vector.tensor_add(
                                T16[:], Tb[:, 0:W2], Tb[:, W2 : 2 * W2]
                            )
                        else:
                            Tb3 = Tb[:].rearrange("p (b x) -> p b x", b=bpd)
                            T16_3 = T16[:].rearrange("p (b x) -> p b x", b=bpd)
                            nc.vector.tensor_add(
                                T16_3[:, :, 0:W2],
                                Tb3[:, :, 0:W2],
                                Tb3[:, :, W2 : 2 * W2],
                            )
                        # optional extra fp16 folds (2 elem/cycle) down to
                        # f16_stop
                        w = W2 // 2
                        T16_3 = (
                            T16[:].rearrange("p (b x) -> p b x", b=bpd)
                            if bpd > 1
                            else None
                        )
                        while w >= f16_stop:
                            if bpd == 1:
                                nc.vector.tensor_add(
                                    T16[:, 0:w], T16[:, 0:w], T16[:, w : 2 * w]
                                )
                            else:
                                nc.vector.tensor_add(
                                    T16_3[:, :, 0:w],
                                    T16_3[:, :, 0:w],
                                    T16_3[:, :, w : 2 * w],
                                )
                            w //= 2
                        for bi in range(bpd):
                            c = ti * bpd + bi
                            base = bi * W2
                            for fi in range(n_folds):
                                is_first = c == 0 and fi == 0
                                is_last = c == GRP - 1 and fi == n_folds - 1
                                o = base + fi * D
                                mm = nc.tensor.matmul(
                                    out=G[:, c : c + 1],
                                    lhsT=T16[:, o : o + D],
                                    rhs=ones16[:],
                                    start=is_first,
                                    stop=is_last,
                                )
                                if start_mm is None:
                                    start_mm = mm
                                elif fi == 0:
                                    add_dep_helper(
                                        mm.ins, start_mm.ins, sync=False,
                                        reason="psum group start first",
                                    )
                                if fi == n_folds - 1:
                                    col_last.append(mm)
                                if is_last:
                                    for prev in col_last[:-1]:
                                        add_dep_helper(
                                            mm.ins, prev.ins, sync=False,
                                            reason="psum group stop last",
                                        )
                    cent_copies.append(
                        nc.scalar.mul(
                            centS[:, g * GRP : (g + 1) * GRP], G[:, 0:GRP], 1.0 / S
                        )
                    )

            if loop_r > 1:
                with tc.For_i(0, loop_r, 1) as _i:
                    emit_bulk()
            else:
                emit_bulk()

        # ---- gather centroids across cores ----
        nc.sync.dma_start(out=loc_cent[:], in_=centS[:])
        if solo:
            nc.sync.dma_start(out=gath[0:D], in_=loc_cent[:])
        else:
            nc.gpsimd.collective_compute(
                "AllGather",
                mybir.AluOpType.bypass,
                replica_groups=[list(range(n_cores))],
                ins=[loc_cent[:]],
                outs=[gath[:]],
            )
        CtEV = consts.tile([D, 2 * B], F16, name="CtEV")
        ct_dma = nc.sync.dma_start(
            out=CtEV[:].rearrange("p (t c j) -> p t c j", t=2, c=n_cores),
            in_=gath.rearrange("(c p) (t j) -> p t c j", c=n_cores, t=2),
        )
        CtE = CtEV[:, 0:B]
        CtV = CtEV[:, B : 2 * B]

        # ---- small pairwise phase (identical on every core) ----
        # PE gate: absorbs the cross-engine deps (cent copies on ACT, the
        # centroid reload DMA) so each later matmul needs <=1 sem wait.
        pe_gate = nc.tensor.nop()
        for ins in cent_copies:
            add_dep_helper(pe_gate.ins, ins.ins, sync=True, reason="pe gate")
        add_dep_helper(pe_gate.ins, ct_dma.ins, sync=True, reason="pe gate dma")
        sp = ctx.enter_context(tc.tile_pool(name="sp" + sfx, bufs=1))
        m2E = sp.tile([D, B], F16, name="m2E")
        m2V = sp.tile([D, B], F16, name="m2V")
        sqE = sp.tile([D, B], F16, name="sqE")
        sqV = sp.tile([D, B], F16, name="sqV")
        nc.vector.tensor_scalar_mul(m2E[:], CtE, -2.0)
        nc.vector.tensor_scalar_mul(m2V[:], CtV, -2.0)
        nc.vector.tensor_mul(sqE[:], CtE, CtE)
        nc.vector.tensor_mul(sqV[:], CtV, CtV)

        # aug tiles: row 0 and row 32 carry {norms, ones}; rest zero (K=64)
        ag_e = sp.tile([64, B], F16, name="ag_e")    # lhsT rows: n_e, 1
        ag_v = sp.tile([64, B], F16, name="ag_v")    # lhsT rows: n_v, 1
        rhs_e = sp.tile([64, B], F16, name="rhs_e")  # rhs rows: 1, n_e
        rhs_v = sp.tile([64, B], F16, name="rhs_v")  # rhs rows: 1, n_v
        acc = sp.tile([128, 3 * n_blk], F32, name="acc")  # per-partition partials
        nc.vector.memset(acc[:], 0.0)

        with tc.tile_pool(name="sps" + sfx, bufs=1, space="PSUM") as sps, tc.tile_pool(
            name="spp" + sfx, bufs=2, space="PSUM"
        ) as spp, tc.tile_pool(name="spf" + sfx, bufs=1, space="PSUM") as spf, tc.tile_pool(
            name="trash" + sfx, bufs=2
        ) as trash_pool:
            def pe_mm(*args, **kwargs):
                mm = nc.tensor.matmul(*args, **kwargs)
                add_dep_helper(mm.ins, pe_gate.ins, sync=False, reason="after gate")
                return mm

            psn = sps.tile([128, 512], F32, name="psn")
            # squared norms at partitions 0 and 32
            pe_mm(out=psn[0:1, 0:B], lhsT=ones16[:], rhs=sqE[:])
            pe_mm(out=psn[32:33, 0:B], lhsT=ones16[:], rhs=sqV[:])
            pe_mm(out=psn[0:1, B : 2 * B], lhsT=ones16[:], rhs=sqV[:])
            pe_mm(out=psn[32:33, B : 2 * B], lhsT=ones16[:], rhs=sqE[:])

            for t, row0_src, row32_src in (
                (ag_e, psn[0:1, 0:B], None),
                (ag_v, psn[0:1, B : 2 * B], None),
                (rhs_e, None, psn[32:33, B : 2 * B]),
                (rhs_v, None, psn[32:33, 0:B]),
            ):
                nc.vector.memset(t[:], 0.0)
                if row0_src is not None:
                    nc.vector.tensor_copy(t[0:1, :], row0_src)
                    nc.vector.memset(t[32:33, :], 1.0)
                else:
                    nc.vector.memset(t[0:1, :], 1.0)
                    nc.vector.tensor_copy(t[32:33, :], row32_src)

            # EV separation: sq = -2 E^T V + (n_e + n_v), then hinge
            for bi, (bs, bn) in enumerate(blks):
                P_evb = spp.tile([128, 512], F32, name="P_ev")
                P_ev = P_evb[:, 0:B]
                pe_mm(
                    out=P_ev[:bn],
                    lhsT=m2E[:, bs : bs + bn],
                    rhs=CtV,
                    start=True,
                    stop=False,
                )
                pe_mm(
                    out=P_ev[:bn],
                    lhsT=ag_e[:, bs : bs + bn],
                    rhs=rhs_v[:],
                    start=False,
                    stop=True,
                )
                dist = trash_pool.tile([128, B], F32, name="dist")
                hin = trash_pool.tile([128, B], F32, name="hin")
                hsq = trash_pool.tile([128, B], F32, name="hsq")
                nc.vector.tensor_scalar_max(P_ev[:bn], P_ev[:bn], 0.0)
                nc.scalar.activation(
                    dist[:bn], P_ev[:bn], mybir.ActivationFunctionType.Sqrt,
                    bias=b_eps[:bn],
                )
                nc.scalar.activation(
                    hin[:bn],
                    dist[:bn],
                    mybir.ActivationFunctionType.Relu,
                    bias=b_margin[:bn],
                    scale=-1.0,
                )
                nc.scalar.activation(
                    hsq[:bn],
                    hin[:bn],
                    mybir.ActivationFunctionType.Square,
                    accum_out=acc[:bn, bi : bi + 1],
                )

            # EE / VV clustering: sq = -2 X^T X + (n + n), relu, sum
            for pi, (m2, Ct, ag, rhs_t) in enumerate(
                ((m2E, CtE, ag_e, rhs_e), (m2V, CtV, ag_v, rhs_v))
            ):
                for bi, (bs, bn) in enumerate(blks):
                    P_clb = spp.tile([128, 512], F32, name="P_cl", tag="P_ev")
                    P_cl = P_clb[:, 0:B]
                    pe_mm(
                        out=P_cl[:bn],
                        lhsT=m2[:, bs : bs + bn],
                        rhs=Ct,
                        start=True,
                        stop=False,
                    )
                    pe_mm(
                        out=P_cl[:bn],
                        lhsT=ag[:, bs : bs + bn],
                        rhs=rhs_t[:],
                        start=False,
                        stop=True,
                    )
                    rel = trash_pool.tile([128, B], F32, name="rel", tag="dist")
                    ci = (1 + pi) * n_blk + bi
                    nc.scalar.activation(
                        rel[:bn],
                        P_cl[:bn],
                        mybir.ActivationFunctionType.Relu,
                        accum_out=acc[:bn, ci : ci + 1],
                    )

            # combine: tot = w_ev*sum(ev) + w_ee*sum(ee) + w_vv*sum(vv)
            t_ev = sp.tile([128, 1], F32, name="t_ev")
            t_ee = sp.tile([128, 1], F32, name="t_ee")
            t_vv = sp.tile([128, 1], F32, name="t_vv")
            tot = sp.tile([128, 1], F32, name="tot")
            for t, base in ((t_ev, 0), (t_ee, n_blk), (t_vv, 2 * n_blk)):
                if n_blk == 1:
                    nc.vector.tensor_copy(t[:], acc[:, base : base + 1])
                else:
                    nc.vector.tensor_add(
                        t[:], acc[:, base : base + 1], acc[:, base + 1 : base + 2]
                    )
                    for k in range(2, n_blk):
                        nc.vector.tensor_add(t[:], t[:], acc[:, base + k : base + k + 1])
            nc.vector.tensor_scalar_mul(tot[:], t_ev[:], w_ev)
            nc.vector.scalar_tensor_tensor(
                tot[:], t_ee[:], w_ee, tot[:],
                op0=mybir.AluOpType.mult, op1=mybir.AluOpType.add,
            )
            nc.vector.scalar_tensor_tensor(
                tot[:], t_vv[:], w_vv, tot[:],
                op0=mybir.AluOpType.mult, op1=mybir.AluOpType.add,
            )
            psFb = spf.tile([128, 512], F32, name="psF")
            psF = psFb[0:1, 0:1]
            pe_mm(out=psF, lhsT=ones_col, rhs=tot[:])
            fin = sp.tile([1, 1], F32, name="fin")
            nc.scalar.copy(fin[:], psF)
            out_dma = nc.sync.dma_start(out=out[:], in_=fin[:])
            return out_dma


def build_nc(B=B, S=S, D=D, n_cores=N_CORES, solo=False,
             bpd=None, bufs=None, loop_r=1, f16_stop=None, n_dma_eng=None,
             serial_unroll=1):
    Bl = B // n_cores
    nc = bacc.Bacc("TRN2", num_devices=n_cores)
    e = nc.dram_tensor("expert_concepts", [Bl, S, D], F32, kind="ExternalInput").ap()
    v = nc.dram_tensor("violator_concepts", [Bl, S, D], F32, kind="ExternalInput").ap()
    out = nc.dram_tensor("out", [1, 1], F32, kind="ExternalOutput").ap()
    loc_cent = nc.dram_tensor("loc_cent", [D, 2 * Bl], F16).ap()
    gath_space = "Local" if solo else "Shared"
    gath = nc.dram_tensor(
        "gath", [n_cores * D, 2 * Bl], F16, addr_space=gath_space
    ).ap()
    with tile.TileContext(nc) as tc:
        dep = None
        for rep in range(serial_unroll):
            dep = _build_body(
                tc, e, v, out, loc_cent, gath, B, S, D, n_cores, solo=solo,
                bpd=bpd if bpd is not None else BPD,
                bufs=bufs if bufs is not None else BUFS,
                loop_r=loop_r,
                f16_stop=f16_stop if f16_stop is not None else F16_STOP,
                n_dma_eng=n_dma_eng,
                start_dep=dep,
                rep=rep,
            )
    nc.compile()
    return nc


def _run(expert_concepts, violator_concepts, **spmd_kwargs):
    expert_concepts = np.ascontiguousarray(expert_concepts, dtype=np.float32)
    violator_concepts = np.ascontiguousarray(violator_concepts, dtype=np.float32)
    assert expert_concepts.shape == (B, S, D)
    assert violator_concepts.shape == (B, S, D)

    nc = build_nc()
    Bl = B // N_CORES
    in_maps = [
        {
            "expert_concepts": expert_concepts[c * Bl : (c + 1) * Bl],
            "violator_concepts": violator_concepts[c * Bl : (c + 1) * Bl],
        }
        for c in range(N_CORES)
    ]
    res = run_bass_kernel_spmd(nc, in_maps, list(range(N_CORES)), **spmd_kwargs)
    return np.float32(res.results[0]["out"][0, 0]), res


def kernel(expert_concepts: np.ndarray, violator_concepts: np.ndarray) -> np.ndarray:
    out, _ = _run(expert_concepts, violator_concepts)
    return out


# revision 35
# speedup vs baseline: 1.0299x; 1.0299x over previous
"""ConceptContrastiveLoss Trainium2 kernel (8-core SPMD, batch-parallel).

Takes FULL inputs expert_concepts/violator_concepts [256, 2048, 128] f32,
returns the scalar loss. Internally shards the batch dim across 8 cores.

Per core (64 MiB of input): each batch item [2048, 128] is DMA'd as one
contiguous 1 MiB transfer into SBUF [128 partitions x 2048] (16 seq rows
per partition).  One VectorE tensor_add folds the halves fp32->fp16
([128, 1024], exact-in-fp32 add, one fp16 rounding per element), then 8
fp16 ones-matmuls on TensorE fold the partitions into a D-major centroid
column accumulated in PSUM.  The tiny [128, 64] fp16 centroid block is
AllGathered; every core then computes the pairwise separation/clustering
terms redundantly in fp16 (-2*X^T*Y matmuls plus a rank-2 norms matmul,
ACT sqrt/relu/square with accum_out reductions in fp32) and core 0's
scalar is returned.

Measured on trn2 (8 cores, long-loop r=257 loop-delta, min/p10/p25
estimators agreeing to ~1 us): the bulk phase runs ~191 us/iteration,
and a DMA-only probe of the same transfer schedule also measures
~191 us => the kernel sits at the per-core HBM bandwidth roofline
(64 MiB / 191 us = 353 GB/s vs the ~358 GB/s HBM-per-NC limit; the SBUF
AXI fabric ceiling of 435 GB/s and the engine budgets - DVE fold
~78 us busy, PE ~55 us busy - are all slack).  Config probes at r=257:
bufs 8 vs 12 vs 16 within 1-3 us (8 best), SWDGE fp32->fp16 cast-DMA
(halves SBUF writes) no faster - confirming the HBM read side binds.
fp16 halves DVE fold work (fp32 fold tree was 2.6 us/tile = 168 us
busy, dangerously close to the roofline) and makes the pairwise-phase
matmuls 1 cycle/row instead of fp32's 4, shrinking the serial tail
after the last DMA (AllGather ~10 us latency-bound + reload + pairwise
+ drain) to ~18 us: full serialized single-shot measures ~209 us
(python-unrolled x17 with cross-repeat DMA gating, loop-delta).

fp16 (10 mantissa bits) over bf16: centroid-sum quantization error ~4x
smaller; all stored magnitudes (pair-sums <~10, centroids ~0.02,
squares ~5e-4) sit comfortably in fp16 normal range.  Measured loss
rel err vs the fp32 jax reference: ~1e-7 (the fp32 output ulp floor).

A remote_dma_broadcast SBUF->SBUF exchange (to replace the ~10 us
AllGather) was probed and abandoned: cross-die transfers routed with an
extra ^2 tpb shift and transfers into cores 0-2 were silently lost
under concurrent all-to-all load; a staggered variant hung the device.
"""

import numpy as np

import concourse.bacc as bacc
import concourse.bass as bass
import concourse.mybir as mybir
import concourse.tile as tile
from concourse.bass_utils import run_bass_kernel_spmd
from concourse.tile import add_dep_helper

F32 = mybir.dt.float32
F16 = mybir.dt.float16

MARGIN = 10.0
ALPHA = 3.0
BETA = 0.3
GAMMA = 0.3

B, S, D = 256, 2048, 128
N_CORES = 8
BPD = 1   # batch items per DMA (DMA size = BPD MiB)
BUFS = 8  # big-tile pool buffers
N_DMA_ENG = 2  # DMA issue streams: 2 = SP+ACT HWDGE, 3 = + gpsimd SWDGE
F16_STOP = 1024  # post-fold width per batch item (1024 => 1 DVE fold, 8 mm)


def _build_body(tc, e, v, out, loc_cent, gath, B, S, D, n_cores, solo=False,
                bpd=BPD, bufs=BUFS, loop_r=1, f16_stop=None, n_dma_eng=None,
                start_dep=None, rep=0, probe=None):
    nc = tc.nc
    if f16_stop is None:
        f16_stop = F16_STOP
    if n_dma_eng is None:
        n_dma_eng = N_DMA_ENG
    Bl = B // n_cores  # local batches per tensor
    J = S // 128       # seq tiles per batch item
    n_pairs = B * (B - 1) // 2
    w_ev = ALPHA / (B * B)
    w_ee = BETA / (2.0 * n_pairs)
    w_vv = GAMMA / (2.0 * n_pairs)
    blks = [(s, min(128, B - s)) for s in range(0, B, 128)]
    n_blk = len(blks)

    from contextlib import ExitStack

    sfx = f"_{rep}" if rep else ""
    with ExitStack() as ctx:
        consts = ctx.enter_context(tc.tile_pool(name="consts" + sfx, bufs=1))
        # preamble-initialized const (no Tile dep => no extra sem wait on
        # matmuls)
        ones_col = nc.const_aps.aps[(F32, 1.0)]
        b_eps = consts.tile([128, 1], F32, name="b_eps")
        nc.vector.memset(b_eps[:], 1e-12)
        b_margin = consts.tile([128, 1], F32, name="b_margin")
        nc.vector.memset(b_margin[:], MARGIN)
        ones16 = consts.tile([128, 1], F16, name="ones16")
        nc.vector.memset(ones16[:], 1.0)
        centS = consts.tile([D, 2 * Bl], F16, name="centS")

        # pairwise-phase tiles whose init doesn't depend on centroids --
        # allocated and memset up front so the fills hide under the bulk DMA
        # aug tiles: row 0 and row 32 carry {norms, ones}; rest zero (K=64)
        ag_e = consts.tile([64, B], F16, name="ag_e")    # lhsT rows: n_e, 1
        ag_v = consts.tile([64, B], F16, name="ag_v")    # lhsT rows: n_v, 1
        rhs_e = consts.tile([64, B], F16, name="rhs_e")  # rhs rows: 1, n_e
        rhs_v = consts.tile([64, B], F16, name="rhs_v")  # rhs rows: 1, n_v
        for t in (ag_e, ag_v):
            nc.vector.memset(t[:], 0.0)
            nc.vector.memset(t[32:33, :], 1.0)
        for t in (rhs_e, rhs_v):
            nc.vector.memset(t[:], 0.0)
            nc.vector.memset(t[0:1, :], 1.0)
        acc = consts.tile([128, 3 * n_blk], F32, name="acc")  # partials
        nc.vector.memset(acc[:], 0.0)
        wts = consts.tile([128, 3 * n_blk], F32, name="wts")  # combine weights
        nc.vector.memset(wts[:, 0:n_blk], w_ev)
        nc.vector.memset(wts[:, n_blk : 2 * n_blk], w_ee)
        nc.vector.memset(wts[:, 2 * n_blk : 3 * n_blk], w_vv)
        if probe == "dma":
            nc.vector.memset(centS[:], 0.0)

        # ---- bulk phase: per-batch centroid sums ----
        # 8 batch columns share one PSUM bank (one accumulation group), so
        # all 8 pool bufs are used exactly once -- no slot reuse, and every
        # matmul carries at most the single DMA/DVE wait it needs.
        cent_copies = []
        GRP = min(8, 2 * Bl)
        n_groups = (2 * Bl + GRP - 1) // GRP
        assert GRP % bpd == 0 and Bl % bpd == 0
        W2 = J * D // 2          # post-fold width per batch item (1024)
        n_folds = f16_stop // D  # matmuls per batch item
        with (
            tc.tile_pool(name="big" + sfx, bufs=bufs) as big_pool,
            tc.tile_pool(name="b16" + sfx, bufs=bufs) as b16_pool,
            tc.tile_pool(name="cps" + sfx, bufs=8, space="PSUM") as cps,
        ):
            dma_engines = [nc.sync, nc.scalar, nc.gpsimd][:n_dma_eng]

            def emit_bulk():
                dma_i = 0
                gated_engines = set()
                for g in range(n_groups):
                    G = cps.tile([128, 512], F32, name="Gacc")
                    start_mm = None
                    col_last = []
                    for ti in range(GRP // bpd):
                        gi0 = g * GRP + ti * bpd
                        t_idx, b0 = divmod(gi0, Bl)
                        src = (e, v)[t_idx]
                        Tb = big_pool.tile([128, bpd * J * D], F32, name="Tb")
                        eng = dma_engines[dma_i % len(dma_engines)]
                        T16 = b16_pool.tile([128, bpd * W2], F16, name="T16")
                        dma_i += 1
                        if bpd == 1:
                            dma = eng.dma_start(
                                out=Tb[:],
                                in_=src[b0].rearrange("(p j) d -> p (j d)", p=128),
                            )
                        else:
                            dma = eng.dma_start(
                                out=Tb[:].rearrange("p (b x) -> p b x", b=bpd),
                                in_=src[b0 : b0 + bpd].rearrange(
                                    "b (p j) d -> p b (j d)", p=128
                                ),
                            )
                        if start_dep is not None and id(eng) not in gated_engines:
                            # HWDGE DMAs are FIFO per issuing engine, so
                            # gating the first DMA on each engine serializes
                            # the whole repeat behind the previous one.
                            gated_engines.add(id(eng))
                            for sd in start_dep:
                                add_dep_helper(
                                    dma.ins, sd.ins, sync=True,
                                    reason="serialize repeats",
                                )
                        if probe == "dma":
                            # consume the tile cheaply so the DMA isn't dead
                            nc.vector.tensor_copy(
                                T16[:, 0:1], Tb[:, 0:1]
                            )
                            continue
                        # one fold: fp32 halves add -> fp16 (exact fp32 add,
                        # single fp16 rounding); covers all bpd batches via
                        # 3D APs to amortize DVE instruction overhead
                        if bpd == 1:
                            nc.vector.tensor_add(
                                T16[:], Tb[:, 0:W2], Tb[:, W2 : 2 * W2]
                            )
                        else:
                            Tb3 = Tb[:].rearrange("p (b x) -> p b x", b=bpd)
                            T16_3 = T16[:].rearrange("p (b x) -> p b x", b=bpd)
                            nc.vector.tensor_add(
                                T16_3[:, :, 0:W2],
                                Tb3[:, :, 0:W2],
                                Tb3[:, :, W2 : 2 * W2],
                            )
                        # optional extra fp16 folds (2 elem/cycle) down to
                        # f16_stop
                        w = W2 // 2
                        T16_3 = (
                            T16[:].rearrange("p (b x) -> p b x", b=bpd)
                            if bpd > 1
                            else None
                        )
                        while w >= f16_stop:
                            if bpd == 1:
                                nc.vector.tensor_add(
                                    T16[:, 0:w], T16[:, 0:w], T16[:, w : 2 * w]
                                )
                            else:
                                nc.vector.tensor_add(
                                    T16_3[:, :, 0:w],
                                    T16_3[:, :, 0:w],
                                    T16_3[:, :, w : 2 * w],
                                )
                            w //= 2
                        for bi in range(bpd):
                            c = ti * bpd + bi
                            base = bi * W2
                            for fi in range(n_folds):
                                is_first = c == 0 and fi == 0
                                is_last = c == GRP - 1 and fi == n_folds - 1
                                o = base + fi * D
                                mm = nc.tensor.matmul(
                                    out=G[:, c : c + 1],
                                    lhsT=T16[:, o : o + D],
                                    rhs=ones16[:],
                                    start=is_first,
                                    stop=is_last,
                                )
                                if start_mm is None:
                                    start_mm = mm
                                elif fi == 0:
                                    add_dep_helper(
                                        mm.ins, start_mm.ins, sync=False,
                                        reason="psum group start first",
                                    )
                                if fi == n_folds - 1:
                                    col_last.append(mm)
                                if is_last:
                                    for prev in col_last[:-1]:
                                        add_dep_helper(
                                            mm.ins, prev.ins, sync=False,
                                            reason="psum group stop last",
                                        )
                    if probe != "dma":
                        cent_copies.append(
                            nc.scalar.mul(
                                centS[:, g * GRP : (g + 1) * GRP], G[:, 0:GRP], 1.0 / S
                            )
                        )

            if loop_r > 1:
                with tc.For_i(0, loop_r, 1) as _i:
                    emit_bulk()
            else:
                emit_bulk()

        # ---- gather centroids across cores ----
        nc.sync.dma_start(out=loc_cent[:], in_=centS[:])
        if solo:
            nc.sync.dma_start(out=gath[0:D], in_=loc_cent[:])
        else:
            nc.gpsimd.collective_compute(
                "AllGather",
                mybir.AluOpType.bypass,
                replica_groups=[list(range(n_cores))],
                ins=[loc_cent[:]],
                outs=[gath[:]],
            )
        CtEV = consts.tile([D, 2 * B], F16, name="CtEV")
        ct_dma = nc.sync.dma_start(
            out=CtEV[:].rearrange("p (t c j) -> p t c j", t=2, c=n_cores),
            in_=gath.rearrange("(c p) (t j) -> p t c j", c=n_cores, t=2),
        )
        CtE = CtEV[:, 0:B]
        CtV = CtEV[:, B : 2 * B]

        # ---- small pairwise phase (identical on every core) ----
        # PE gate: absorbs the cross-engine deps (cent copies on ACT, the
        # centroid reload DMA) so each later matmul needs <=1 sem wait.
        pe_gate = nc.tensor.nop()
        for ins in cent_copies:
            add_dep_helper(pe_gate.ins, ins.ins, sync=True, reason="pe gate")
        add_dep_helper(pe_gate.ins, ct_dma.ins, sync=True, reason="pe gate dma")
        sp = ctx.enter_context(tc.tile_pool(name="sp" + sfx, bufs=1))
        m2E = sp.tile([D, B], F16, name="m2E")
        m2V = sp.tile([D, B], F16, name="m2V")
        sqE = sp.tile([D, B], F16, name="sqE")
        sqV = sp.tile([D, B], F16, name="sqV")
        nc.vector.tensor_scalar_mul(m2E[:], CtE, -2.0)
        nc.vector.tensor_scalar_mul(m2V[:], CtV, -2.0)
        nc.vector.tensor_mul(sqE[:], CtE, CtE)
        nc.vector.tensor_mul(sqV[:], CtV, CtV)

        with tc.tile_pool(name="sps" + sfx, bufs=1, space="PSUM") as sps, tc.tile_pool(
            name="spp" + sfx, bufs=4, space="PSUM"
        ) as spp, tc.tile_pool(name="spf" + sfx, bufs=1, space="PSUM") as spf, tc.tile_pool(
            name="trash" + sfx, bufs=2
        ) as trash_pool:
            def pe_mm(*args, **kwargs):
                mm = nc.tensor.matmul(*args, **kwargs)
                add_dep_helper(mm.ins, pe_gate.ins, sync=False, reason="after gate")
                return mm

            psn = sps.tile([128, 512], F32, name="psn")
            # squared norms at partitions 0 and 32
            pe_mm(out=psn[0:1, 0:B], lhsT=ones16[:], rhs=sqE[:])
            pe_mm(out=psn[32:33, 0:B], lhsT=ones16[:], rhs=sqV[:])
            pe_mm(out=psn[0:1, B : 2 * B], lhsT=ones16[:], rhs=sqV[:])
            pe_mm(out=psn[32:33, B : 2 * B], lhsT=ones16[:], rhs=sqE[:])

            # only the norm rows depend on centroids; zeros/ones pre-filled
            nc.vector.tensor_copy(ag_e[0:1, :], psn[0:1, 0:B])
            nc.vector.tensor_copy(ag_v[0:1, :], psn[0:1, B : 2 * B])
            nc.vector.tensor_copy(rhs_e[32:33, :], psn[32:33, B : 2 * B])
            nc.vector.tensor_copy(rhs_v[32:33, :], psn[32:33, 0:B])

            # EV separation: sq = -2 E^T V + (n_e + n_v), then hinge
            for bi, (bs, bn) in enumerate(blks):
                P_evb = spp.tile([128, 512], F32, name="P_ev")
                P_ev = P_evb[:, 0:B]
                pe_mm(
                    out=P_ev[:bn],
                    lhsT=m2E[:, bs : bs + bn],
                    rhs=CtV,
                    start=True,
                    stop=False,
                )
                pe_mm(
                    out=P_ev[:bn],
                    lhsT=ag_e[:, bs : bs + bn],
                    rhs=rhs_v[:],
                    start=False,
                    stop=True,
                )
                dist = trash_pool.tile([128, B], F32, name="dist")
                hin = trash_pool.tile([128, B], F32, name="hin")
                hsq = trash_pool.tile([128, B], F32, name="hsq")
                nc.vector.tensor_scalar_max(P_ev[:bn], P_ev[:bn], 0.0)
                nc.scalar.activation(
                    dist[:bn], P_ev[:bn], mybir.ActivationFunctionType.Sqrt,
                    bias=b_eps[:bn],
                )
                nc.scalar.activation(
                    hin[:bn],
                    dist[:bn],
                    mybir.ActivationFunctionType.Relu,
                    bias=b_margin[:bn],
                    scale=-1.0,
                )
                nc.scalar.activation(
                    hsq[:bn],
                    hin[:bn],
                    mybir.ActivationFunctionType.Square,
                    accum_out=acc[:bn, bi : bi + 1],
                )

            # EE / VV clustering: sq = -2 X^T X + (n + n), relu, sum
            for pi, (m2, Ct, ag, rhs_t) in enumerate(
                ((m2E, CtE, ag_e, rhs_e), (m2V, CtV, ag_v, rhs_v))
            ):
                for bi, (bs, bn) in enumerate(blks):
                    P_clb = spp.tile([128, 512], F32, name="P_cl", tag="P_ev")
                    P_cl = P_clb[:, 0:B]
                    pe_mm(
                        out=P_cl[:bn],
                        lhsT=m2[:, bs : bs + bn],
                        rhs=Ct,
                        start=True,
                        stop=False,
                    )
                    pe_mm(
                        out=P_cl[:bn],
                        lhsT=ag[:, bs : bs + bn],
                        rhs=rhs_t[:],
                        start=False,
                        stop=True,
                    )
                    rel = trash_pool.tile([128, B], F32, name="rel", tag="dist")
                    ci = (1 + pi) * n_blk + bi
                    nc.scalar.activation(
                        rel[:bn],
                        P_cl[:bn],
                        mybir.ActivationFunctionType.Relu,
                        accum_out=acc[:bn, ci : ci + 1],
                    )

            # combine: tot[p] = sum_c acc[p, c] * wts[p, c] in one DVE op
            tot = sp.tile([128, 1], F32, name="tot")
            junk = sp.tile([128, 3 * n_blk], F32, name="junk")
            nc.vector.tensor_tensor_reduce(
                out=junk[:],
                in0=acc[:],
                in1=wts[:],
                scale=1.0,
                scalar=0.0,
                op0=mybir.AluOpType.mult,
                op1=mybir.AluOpType.add,
                accum_out=tot[:],
            )
            psFb = spf.tile([128, 512], F32, name="psF")
            psF = psFb[0:1, 0:1]
            pe_mm(out=psF, lhsT=ones_col, rhs=tot[:])
            fin = sp.tile([1, 1], F32, name="fin")
            nc.scalar.copy(fin[:], psF)
            out_dma = nc.sync.dma_start(out=out[:], in_=fin[:])
            return [out_dma]


def build_nc(B=B, S=S, D=D, n_cores=N_CORES, solo=False,
             bpd=None, bufs=None, loop_r=1, f16_stop=None, n_dma_eng=None,
             serial_unroll=1, probe=None):
    Bl = B // n_cores
    nc = bacc.Bacc("TRN2", num_devices=n_cores)
    e = nc.dram_tensor("expert_concepts", [Bl, S, D], F32, kind="ExternalInput").ap()
    v = nc.dram_tensor("violator_concepts", [Bl, S, D], F32, kind="ExternalInput").ap()
    out = nc.dram_tensor("out", [1, 1], F32, kind="ExternalOutput").ap()
    loc_cent = nc.dram_tensor("loc_cent", [D, 2 * Bl], F16).ap()
    gath_space = "Local" if solo else "Shared"
    gath = nc.dram_tensor(
        "gath", [n_cores * D, 2 * Bl], F16, addr_space=gath_space
    ).ap()
    with tile.TileContext(nc) as tc:
        dep = None
        for rep in range(serial_unroll):
            dep = _build_body(
                tc, e, v, out, loc_cent, gath, B, S, D, n_cores, solo=solo,
                bpd=bpd if bpd is not None else BPD,
                bufs=bufs if bufs is not None else BUFS,
                loop_r=loop_r,
                f16_stop=f16_stop if f16_stop is not None else F16_STOP,
                n_dma_eng=n_dma_eng,
                start_dep=dep,
                rep=rep,
                probe=probe,
            )
    nc.compile()
    return nc


def _run(expert_concepts, violator_concepts, **spmd_kwargs):
    expert_concepts = np.ascontiguousarray(expert_concepts, dtype=np.float32)
    violator_concepts = np.ascontiguousarray(violator_concepts, dtype=np.float32)
    assert expert_concepts.shape == (B, S, D)
    assert violator_concepts.shape == (B, S, D)

    nc = build_nc()
    Bl = B // N_CORES
    in_maps = [
        {
            "expert_concepts": expert_concepts[c * Bl : (c + 1) * Bl],
            "violator_concepts": violator_concepts[c * Bl : (c + 1) * Bl],
        }
        for c in range(N_CORES)
    ]
    res = run_bass_kernel_spmd(nc, in_maps, list(range(N_CORES)), **spmd_kwargs)
    return np.float32(res.results[0]["out"][0, 0]), res


def kernel(expert_concepts: np.ndarray, violator_concepts: np.ndarray) -> np.ndarray:
    out, _ = _run(expert_concepts, violator_concepts)
    return out
